# revision 1
# baseline (speedup 1.0000x reference)
"""AllSetTransformerLayer distributed Trainium2 kernel (8 NeuronCores).

Strategy (hardcoded for N=20000 nodes, M=5003 hyperedges, E=320000, C=256,
HID=512, HEADS=4, QN=1):

- QN=1 lets the attention logits fold to logits = x @ (K[h] @ Q[h]) per head,
  and segment softmax folds to attn = u[src]/denom[tgt] with u = exp(logits)
  (constant-shift-invariant; logits are O(1) so no max subtraction needed).
- Each core owns a node shard (2500) and a hyperedge shard (626; padded
  global hyperedge count 5008). Per block it computes its shard's
  y = [xV * u_perhead | u | pad] rows (640 cols, bf16), AllGathers the y
  table, then dma_gathers the 640-col rows of its targets' (capacity-padded)
  source lists and reduces 64 (block1) / 16 (block2) slots per target with
  one-hot strip matmuls accumulating in PSUM. Normalize + seed + LN + MLP +
  LN + relu runs per 128-target tile, all on-chip.
- Pad gather slots point at a dedicated zero row appended to each y table.
"""
import sys
import os
import numpy as np

for _p in ("/opt/trn_rl_repo", "/root/.axon_site/_ro/trn_rl_repo"):
    if os.path.isdir(_p) and _p not in sys.path:
        sys.path.insert(0, _p)

import ml_dtypes

BF16 = ml_dtypes.bfloat16

N_NODES, N_HEDGES, E = 20000, 5003, 320000
IN_C, HID, HEADS, DH = 256, 512, 4, 128
N_CORES = 8
NPC = N_NODES // N_CORES            # 2500 nodes/core
HPC = -(-N_HEDGES // N_CORES)       # 626 hedges/core
MPAD = HPC * N_CORES                # 5008
YC = 640                            # y row: 512 vals + 4 u + 124 pad (bf16)
CHUNK = 2048                        # edges per dma_gather
C1, C2 = 64, 16                     # segment capacities (block1 / block2)
ZROW1, ZROW2 = N_NODES, MPAD        # zero-row indices in the y tables
AG1_SPLIT = 1280                    # per-core rows in the first AG1 half
AG2_SPLIT = 384                     # per-core rows in the first AG2 half
K2CH = 40                           # dense block2: hedge K-chunks (40*128=5120)
G2T = 14                            # block2 node-tiles via dma_gather
D2T = 6                             # block2 node-tiles via dense incidence matmul
B1_CALLS = 16                       # block1 gather calls (tiles 0..3; tile 4 is dense)
K1CH = 157                          # dense block1: node K-chunks (157*128=20096)
B2_CALLS = NPC * C2 // CHUNK + (1 if (NPC * C2) % CHUNK else 0)   # 20
B1_PTILES = -(-HPC // 128)          # 5 psum tiles of 128 hedges
B2_PTILES = -(-NPC // 128)          # 20 psum tiles of 128 nodes
LN_EPS = 1e-5

_CACHE = {}


# ----------------------------------------------------------------- host prep

def _fold_qv(K, Q):
    return np.einsum('hcd,hd->ch', K, Q[:, 0, :]).astype(np.float32)


def _concat_heads(V):
    H, C, D = V.shape
    return np.ascontiguousarray(np.transpose(V, (1, 0, 2)).reshape(C, H * D)).astype(np.float32)


def _bcast(v):
    return np.ascontiguousarray(np.broadcast_to(np.asarray(v, np.float32)[None, :], (128, HID)))


def _build_slots(tgt, src, cap, zero_row, tgt_lo, tgt_hi, total_pad):
    """Slot list for targets [tgt_lo, tgt_hi): int32 [(hi-lo)*cap], padded to
    total_pad with zero_row."""
    n_take = tgt_hi - tgt_lo
    mask = (tgt >= tgt_lo) & (tgt < tgt_hi)
    t = tgt[mask].astype(np.int64) - tgt_lo
    s = src[mask].astype(np.int64)
    order = np.argsort(t, kind='stable')
    t = t[order]
    s = s[order]
    counts = np.bincount(t, minlength=n_take)
    if counts.max(initial=0) > cap:
        raise ValueError(f"max segment degree {counts.max()} exceeds capacity {cap}")
    out = np.full(total_pad, zero_row, dtype=np.int32)
    starts = np.concatenate([[0], np.cumsum(counts)[:-1]])
    pos = np.arange(len(t)) - starts[t]
    out[t * cap + pos] = s
    return out


def _ag_remap(s, per_core, split, n_real):
    """Remap global source ids to split-AllGather table rows; ids >= n_real
    (zero-row pads) pass through."""
    s = s.copy()
    real = s < n_real
    v = s[real]
    c = v // per_core
    l = v % per_core
    lo = l < split
    out = np.where(lo, c * split + l,
                   split * N_CORES + c * (per_core - split) + (l - split))
    s[real] = out
    return s


def _wrap_idx16(a):
    """[total] int32 -> [16, total//16] int16, element i at [i%16, i//16]."""
    assert a.max(initial=0) < 32768 and len(a) % 16 == 0
    return np.ascontiguousarray(np.tile(a.reshape(-1, 16).T.astype(np.int16), (8, 1)))


def _build_strip(cap):
    W = np.zeros((128, 256), dtype=np.float32)
    p = np.arange(128)
    W[p, 127 + p // cap] = 1.0
    return W.astype(BF16)


def _host_prep(inputs):
    x0 = np.asarray(inputs['x_0'], np.float32)
    node_idx = np.asarray(inputs['node_idx']).astype(np.int64)
    hedge_idx = np.asarray(inputs['hedge_idx']).astype(np.int64)

    shared = {
        'V1': _concat_heads(np.asarray(inputs['ve_V'], np.float32)),
        'qv1': _fold_qv(np.asarray(inputs['ve_K'], np.float32), np.asarray(inputs['ve_Q'], np.float32)),
        'V2': _concat_heads(np.asarray(inputs['ev_V'], np.float32)),
        'qv2': _fold_qv(np.asarray(inputs['ev_K'], np.float32), np.asarray(inputs['ev_Q'], np.float32)),
        'W11': np.ascontiguousarray(np.asarray(inputs['ve_w1'], np.float32).T).astype(BF16),
        'W12': np.ascontiguousarray(np.asarray(inputs['ve_w2'], np.float32).T).astype(BF16),
        'W21': np.ascontiguousarray(np.asarray(inputs['ev_w1'], np.float32).T).astype(BF16),
        'W22': np.ascontiguousarray(np.asarray(inputs['ev_w2'], np.float32).T).astype(BF16),
        'seed1': _bcast(np.asarray(inputs['ve_Q'], np.float32)[:, 0, :].reshape(-1)),
        'l0g1': _bcast(inputs['ve_ln0_g']), 'l0b1': _bcast(inputs['ve_ln0_b']),
        'b11': _bcast(inputs['ve_b1']), 'b12': _bcast(inputs['ve_b2']),
        'l1g1': _bcast(inputs['ve_ln1_g']), 'l1b1': _bcast(inputs['ve_ln1_b']),
        'seed2': _bcast(np.asarray(inputs['ev_Q'], np.float32)[:, 0, :].reshape(-1)),
        'l0g2': _bcast(inputs['ev_ln0_g']), 'l0b2': _bcast(inputs['ev_ln0_b']),
        'b21': _bcast(inputs['ev_b1']), 'b22': _bcast(inputs['ev_b2']),
        'l1g2': _bcast(inputs['ev_ln1_g']), 'l1b2': _bcast(inputs['ev_ln1_b']),
        'strip1': _build_strip(C1),
        'strip2': _build_strip(C2),
        'ident': np.eye(128, dtype=np.float32),
        'identb': np.eye(128, dtype=BF16),
    }

    # pack the shared weight tensors
    fw = np.concatenate([
        shared['V1'][0:128], shared['V1'][128:256],
        shared['V2'][0:128], shared['V2'][128:256],
        shared['V2'][256:384], shared['V2'][384:512],
        shared['qv1'][0:128], shared['qv1'][128:256],
        shared['qv2'][0:128], shared['qv2'][128:256],
        shared['qv2'][256:384], shared['qv2'][384:512],
        shared['ident'],
    ], axis=1).astype(np.float32)
    bc_names = ['seed1', 'l0g1', 'l0b1', 'b11', 'b12', 'l1g1', 'l1b1',
                'seed2', 'l0g2', 'l0b2', 'b21', 'b22', 'l1g2', 'l1b2']
    bcst = np.concatenate([shared[k] for k in bc_names], axis=1).astype(np.float32)
    wmlp = np.concatenate(
        [shared[nm][128 * k:128 * (k + 1)]
         for nm in ('W11', 'W12', 'W21', 'W22') for k in range(4)],
        axis=1).astype(BF16)
    bfw = np.concatenate([shared['strip1'], shared['strip2'], shared['identb']],
                         axis=1).astype(BF16)
    packed = {
        'fw': np.ascontiguousarray(fw),
        'bcst': np.ascontiguousarray(bcst),
        'wmlp': np.ascontiguousarray(wmlp),
        'bfw': np.ascontiguousarray(bfw),
    }
    in_maps = []
    for c in range(N_CORES):
        m = dict(packed)
        xt = x0[c * NPC:(c + 1) * NPC].T  # [256, NPC]
        m['x0T'] = np.ascontiguousarray(np.concatenate([xt[0:128], xt[128:256]], axis=1))
        s1 = _build_slots(hedge_idx, node_idx, C1, ZROW1,
                          c * HPC, c * HPC + 512, B1_CALLS * CHUNK)
        # dense incidence for local hedges [512, 626) (block1 tile 4)
        mask1 = (hedge_idx >= c * HPC + 512) & (hedge_idx < c * HPC + HPC) \
            & (hedge_idx < N_HEDGES)
        inc1 = np.zeros((K1CH * 128, 128), np.float32)
        np.add.at(inc1, (node_idx[mask1], hedge_idx[mask1] - (c * HPC + 512)), 1.0)
        m['b1t'] = np.ascontiguousarray(
            inc1.reshape(K1CH, 128, 128).transpose(1, 0, 2)
            .reshape(128, K1CH * 128)).astype(BF16)
        # block2 gather tiles cover local nodes [0, G2T*128)
        s2 = _build_slots(node_idx, hedge_idx, C2, ZROW2,
                          c * NPC, c * NPC + G2T * 128, G2T * CHUNK)
        m['idx'] = np.ascontiguousarray(
            np.concatenate([_wrap_idx16(s1), _wrap_idx16(s2)], axis=1))
        # dense incidence for local nodes [G2T*128, 2560) (tiles 14..19)
        mask = (node_idx >= c * NPC + G2T * 128) & (node_idx < (c + 1) * NPC)
        hh = hedge_idx[mask]
        ll = node_idx[mask] - (c * NPC + G2T * 128)
        inc = np.zeros((K2CH * 128, D2T * 128), np.float32)
        np.add.at(inc, (hh, ll), 1.0)
        b2t = inc.reshape(K2CH, 128, D2T, 128).transpose(2, 1, 0, 3)
        m['b2t'] = np.ascontiguousarray(
            b2t.reshape(D2T * 128, K2CH * 128)).astype(BF16)
        in_maps.append(m)
    return in_maps


# ----------------------------------------------------------------- builder

def _build(trivial_ln=(True,) * 4, trivial_b=(True,) * 2):
    from concourse import bacc, tile, mybir
    from concourse.bass import _add_dep_helper

    dt = mybir.dt
    Alu = mybir.AluOpType
    Act = mybir.ActivationFunctionType
    F32, F32R, BF, I16 = dt.float32, dt.float32r, dt.bfloat16, dt.int16

    nc = bacc.Bacc("TRN2", target_bir_lowering=False, debug=False,
                   num_devices=N_CORES)

    def din(name, shape, dtype=F32):
        return nc.dram_tensor(name, shape, dtype, kind="ExternalInput")

    bc_names = ['seed1', 'l0g1', 'l0b1', 'b11', 'b12', 'l1g1', 'l1b1',
                'seed2', 'l0g2', 'l0b2', 'b21', 'b22', 'l1g2', 'l1b2']
    # packed inputs: few large DMAs instead of ~40 small ones
    x0T_d = din('x0T', [128, 2 * NPC])                       # 2 partition-chunks
    fw_d = din('fw', [128, 6 * HID + 6 * HEADS + 128])       # V1(2)+V2(4)+qv+ident
    bcst_d = din('bcst', [128, len(bc_names) * HID])
    wmlp_d = din('wmlp', [128, 16 * HID], BF)
    bfw_d = din('bfw', [128, 2 * 256 + 128], BF)             # strips + identb
    idx_d = din('idx', [128, (B1_CALLS + G2T) * CHUNK // 16], I16)
    b2t_d = din('b2t', [D2T * 128, K2CH * 128], BF)
    b1t_d = din('b1t', [128, K1CH * 128], BF)

    out_d = nc.dram_tensor('out', [NPC, HID], F32, kind="ExternalOutput")

    y1loc = nc.dram_tensor('y1loc', [NPC, YC], BF)
    y1full = nc.dram_tensor('y1full', [K1CH * 128, YC], BF, addr_space="Shared")
    y2loc = nc.dram_tensor('y2loc', [HPC, YC], BF)
    y2full = nc.dram_tensor('y2full', [K2CH * 128, YC], BF, addr_space="Shared")

    rg = [list(range(N_CORES))]

    with tile.TileContext(nc) as tc:
        wp = tc.alloc_tile_pool(name="wp", bufs=1)
        sp = tc.alloc_tile_pool(name="sp", bufs=2)
        st = tc.alloc_tile_pool(name="st", bufs=4)
        segp = tc.alloc_tile_pool(name="segp", bufs=2, space="PSUM")
        mmp = tc.alloc_tile_pool(name="mmp", bufs=2, space="PSUM")
        tp = tc.alloc_tile_pool(name="tp", bufs=2, space="PSUM")
        xp = tc.alloc_tile_pool(name="xp", bufs=1)
        gp = tc.alloc_tile_pool(name="gp", bufs=2)

        # ---- resident weights/tables (one DMA per packed tensor)
        x0T_t = xp.tile([128, 2 * NPC], F32, name="x0T_t", tag="x0T_t")
        nc.sync.dma_start(out=x0T_t[:], in_=x0T_d[:])
        fw_t = wp.tile([128, 6 * HID + 6 * HEADS + 128], F32, name="fw_t", tag="fw_t")
        nc.sync.dma_start(out=fw_t[:], in_=fw_d[:])
        # load only the broadcast tensors the graph will reference
        _bc_cache = {}

        def get_bc(name):
            if name not in _bc_cache:
                i = bc_names.index(name)
                t = wp.tile([128, HID], F32, name=f"bc_{name}", tag=f"bc_{name}")
                nc.sync.dma_start(out=t[:], in_=bcst_d[:, i * HID:(i + 1) * HID])
                _bc_cache[name] = t
            return _bc_cache[name]
        wmlp_t = wp.tile([128, 16 * HID], BF, name="wmlp_t", tag="wmlp_t")
        nc.sync.dma_start(out=wmlp_t[:], in_=wmlp_d[:])
        bfw_t = wp.tile([128, 2 * 256 + 128], BF, name="bfw_t", tag="bfw_t")
        nc.sync.dma_start(out=bfw_t[:], in_=bfw_d[:])
        idx_t = xp.tile([128, B1_CALLS * CHUNK // 16], I16,
                        name="idx_t", tag="idx_t")
        nc.sync.dma_start(out=idx_t[:], in_=idx_d[:, :B1_CALLS * CHUNK // 16])
        idx2_t = wp.tile([128, G2T * CHUNK // 16], I16, name="idx2_t", tag="idx2_t")
        nc.sync.dma_start(out=idx2_t[:], in_=idx_d[:, B1_CALLS * CHUNK // 16:])

        x0T = [x0T_t[:, k * NPC:(k + 1) * NPC] for k in range(2)]
        V1 = [fw_t[:, k * HID:(k + 1) * HID] for k in range(2)]
        V2 = [fw_t[:, (2 + k) * HID:(3 + k) * HID] for k in range(4)]
        qv1 = [fw_t[:, 6 * HID + k * HEADS:6 * HID + (k + 1) * HEADS] for k in range(2)]
        qv2 = [fw_t[:, 6 * HID + (2 + k) * HEADS:6 * HID + (3 + k) * HEADS] for k in range(4)]
        ident = fw_t[:, 6 * HID + 6 * HEADS:6 * HID + 6 * HEADS + 128]
        W = {}
        for i, nm in enumerate(('W11', 'W12', 'W21', 'W22')):
            W[nm] = [wmlp_t[:, (4 * i + k) * HID:(4 * i + k + 1) * HID] for k in range(4)]
        strip1 = bfw_t[:, 0:256]
        strip2 = bfw_t[:, 256:512]
        identb = bfw_t[:, 512:640]
        idx1 = idx_t[:, 0:B1_CALLS * CHUNK // 16]
        idx2 = idx2_t[:, :]

        eps_t = wp.tile([128, 1], F32, name="eps_t", tag="eps_t")
        nc.vector.memset(eps_t[:], LN_EPS)

        # zero rows of the gather tables
        ztile = wp.tile([128, YC], BF, name="ztile", tag="ztile")
        nc.vector.memset(ztile[:], 0.0)
        z1 = nc.sync.dma_start(out=y1full[N_NODES:K1CH * 128, :],
                               in_=ztile[:K1CH * 128 - N_NODES, :])
        z2 = nc.sync.dma_start(out=y2full[MPAD:K2CH * 128, :],
                               in_=ztile[:K2CH * 128 - MPAD, :])

        # persistent transposed x1 (block1 output), 4 feature chunks
        x1T = [wp.tile([128, B1_PTILES * 128], F32, name=f"x1T{k}", tag=f"x1T{k}") for k in range(4)]

        # ---------------- helpers
        def emit_y(n_rows, n_kc, xT_tiles, rt, Vt, qvt, yloc, row0):
            """y/u production for one row tile: y row block -> yloc[row0:...]."""
            rows = n_rows
            py = mmp.tile([128, HID], F32, name="mm", tag="mm")
            plg = tp.tile([128, 128], F32, name="tp", tag="tp")
            for kc in range(n_kc):
                lhsT = xT_tiles[kc][:, rt * 128: rt * 128 + rows]
                nc.tensor.matmul(py[:rows, :], lhsT, Vt[kc][:],
                                 start=(kc == 0), stop=(kc == n_kc - 1))
            for kc in range(n_kc):
                lhsT = xT_tiles[kc][:, rt * 128: rt * 128 + rows]
                nc.tensor.matmul(plg[:rows, :HEADS], lhsT, qvt[kc][:],
                                 start=(kc == 0), stop=(kc == n_kc - 1))
            u = st.tile([128, HEADS], F32, name="u", tag="u")
            nc.scalar.activation(u[:rows, :], plg[:rows, :HEADS], Act.Exp)
            ysb = sp.tile([128, YC], BF, name="ysb", tag="ysb")
            for h in range(HEADS):
                nc.vector.tensor_scalar_mul(ysb[:rows, h * DH:(h + 1) * DH],
                                            py[:rows, h * DH:(h + 1) * DH],
                                            u[:rows, h:h + 1])
            nc.vector.tensor_copy(ysb[:rows, HID:HID + HEADS], u[:rows, :])
            nc.vector.memset(ysb[:rows, HID + HEADS:], 0.0)
            return nc.scalar.dma_start(out=yloc[row0:row0 + rows, :], in_=ysb[:rows, :])

        def emit_ln(x_sb, rows, g_name, b_name, out_sb, trivial):
            musum = st.tile([128, 1], F32, name="musum", tag="musum")
            nc.vector.tensor_reduce(musum[:rows, :], x_sb[:rows, :],
                                    mybir.AxisListType.X, Alu.add)
            negmu = st.tile([128, 1], F32, name="negmu", tag="negmu")
            nc.vector.tensor_scalar_mul(negmu[:rows, :], musum[:rows, :], -1.0 / HID)
            sq = sp.tile([128, HID], F32, name="lnsq", tag="lnsq")
            sqs = st.tile([128, 1], F32, name="sqs", tag="sqs")
            nc.scalar.activation(sq[:rows, :], x_sb[:rows, :], Act.Square,
                                 bias=negmu[:rows, :], accum_out=sqs[:rows, :])
            sstd = st.tile([128, 1], F32, name="sstd", tag="sstd")
            nc.scalar.activation(sstd[:rows, :], sqs[:rows, :], Act.Sqrt,
                                 bias=eps_t[:rows, :], scale=1.0 / HID)
            rstd = st.tile([128, 1], F32, name="rstd", tag="rstd")
            nc.vector.reciprocal(rstd[:rows, :], sstd[:rows, :])
            nmr = st.tile([128, 1], F32, name="nmr", tag="nmr")
            nc.vector.tensor_mul(nmr[:rows, :], negmu[:rows, :], rstd[:rows, :])
            # out = (x - mu) * rstd, fused on ACT: Identity(x*rstd + negmu*rstd)
            nc.scalar.activation(out_sb[:rows, :], x_sb[:rows, :], Act.Identity,
                                 bias=nmr[:rows, :], scale=rstd[:rows, 0:1])
            if not trivial:
                nc.vector.tensor_mul(out_sb[:rows, :], out_sb[:rows, :],
                                     get_bc(g_name)[:rows, :])
                nc.vector.tensor_add(out_sb[:rows, :], out_sb[:rows, :],
                                     get_bc(b_name)[:rows, :])

        def emit_post(pseg, rows, blk, out_sb):
            """psum [128, YC] -> normalized+LN+MLP+LN+relu -> out_sb f32 [128, HID]."""
            sfx = str(blk)
            seedb = get_bc('seed' + sfx)
            recip = st.tile([128, HEADS], F32, name="recip", tag="recip")
            dtmp = st.tile([128, HEADS], F32, name="dtmp", tag="dtmp")
            nc.vector.tensor_scalar_add(dtmp[:rows, :], pseg[:rows, HID:HID + HEADS],
                                        1e-30)
            nc.vector.reciprocal(recip[:rows, :], dtmp[:rows, :])
            s_sb = sp.tile([128, HID], F32, name="s", tag="s")
            for h in range(HEADS):
                nc.vector.scalar_tensor_tensor(
                    s_sb[:rows, h * DH:(h + 1) * DH],
                    pseg[:rows, h * DH:(h + 1) * DH],
                    recip[:rows, h:h + 1],
                    seedb[:rows, h * DH:(h + 1) * DH],
                    Alu.mult, Alu.add)
            xn = sp.tile([128, HID], F32, name="xn", tag="xn")
            emit_ln(s_sb, rows, 'l0g' + sfx, 'l0b' + sfx, xn,
                    trivial_ln[0 if sfx == '1' else 2])
            # MLP layer 1
            hps = mmp.tile([128, HID], F32, name="mm", tag="mm")
            for kc in range(4):
                tt = tp.tile([128, 128], F32, name="tp", tag="tp")
                nc.tensor.transpose(tt[:, :], xn[:, kc * 128:(kc + 1) * 128], ident)
                xnT = sp.tile([128, 128], BF, name="xnT", tag="xnT")
                nc.vector.tensor_copy(xnT[:, :], tt[:, :])
                nc.tensor.matmul(hps[:, :], xnT[:], W['W' + sfx + '1'][kc][:],
                                 start=(kc == 0), stop=(kc == 3))
            h_sb = sp.tile([128, HID], BF, name="hsb", tag="hsb")
            if trivial_b[0 if sfx == '1' else 1]:
                nc.scalar.activation(h_sb[:rows, :], hps[:rows, :], Act.Relu)
            else:
                htmp = sp.tile([128, HID], F32, name="lnsq", tag="lnsq")
                nc.vector.tensor_add(htmp[:rows, :], hps[:rows, :],
                                     get_bc('b' + sfx + '1')[:rows, :])
                nc.vector.tensor_scalar_max(h_sb[:rows, :], htmp[:rows, :], 0.0)
            # MLP layer 2
            fps = mmp.tile([128, HID], F32, name="mm", tag="mm")
            for kc in range(4):
                tt = tp.tile([128, 128], BF, name="tp", tag="tp")
                nc.tensor.transpose(tt[:, :], h_sb[:, kc * 128:(kc + 1) * 128], identb)
                hT = sp.tile([128, 128], BF, name="xnT", tag="xnT")
                nc.vector.tensor_copy(hT[:, :], tt[:, :])
                nc.tensor.matmul(fps[:, :], hT[:], W['W' + sfx + '2'][kc][:],
                                 start=(kc == 0), stop=(kc == 3))
            z = sp.tile([128, HID], F32, name="z", tag="z")
            if trivial_b[0 if sfx == '1' else 1]:
                nc.vector.scalar_tensor_tensor(z[:rows, :], fps[:rows, :], 0.0,
                                               xn[:rows, :], Alu.max, Alu.add)
            else:
                ftmp = sp.tile([128, HID], F32, name="lnsq", tag="lnsq")
                nc.vector.tensor_add(ftmp[:rows, :], fps[:rows, :],
                                     get_bc('b' + sfx + '2')[:rows, :])
                nc.vector.scalar_tensor_tensor(z[:rows, :], ftmp[:rows, :], 0.0,
                                               xn[:rows, :], Alu.max, Alu.add)
            zn = sp.tile([128, HID], F32, name="s", tag="s")
            emit_ln(z, rows, 'l1g' + sfx, 'l1b' + sfx, zn,
                    trivial_ln[1 if sfx == '1' else 3])
            nc.scalar.activation(out_sb[:rows, :], zn[:rows, :], Act.Relu)

        # ---------------- block 1: y1 production
        y1_writes = []
        for rt in range(B2_PTILES):  # 20 row tiles over NPC nodes
            rows = min(128, NPC - rt * 128)
            y1_writes.append(emit_y(rows, 2, x0T, rt, V1, qv1, y1loc, rt * 128))

        cc1a = cc1b = nc.gpsimd.collective_compute(
            "AllGather", Alu.bypass, replica_groups=rg,
            ins=[y1loc[:, :]], outs=[y1full[0:N_NODES, :]])
        for w in y1_writes:
            _add_dep_helper(cc1a.ins, w.ins, sync=True, reason="ag1 after y1 writes")

        # ---------------- block 1: gather + segment sums + post -> x1T
        y2_writes = []
        SLOTS = CHUNK // 128
        CPT1 = 128 * C1 // CHUNK

        # dense hedge-tile 4 (local hedges 512..625) on PE, overlapping the
        # gather train below
        pseg4 = segp.tile([128, YC], F32, name="seg4", tag="seg")
        NG1 = 8  # node chunks per streamed group
        for g_ in range((K1CH + NG1 - 1) // NG1):
            lo = g_ * NG1
            n = min(NG1, K1CH - lo)
            yt = gp.tile([128, NG1, YC], BF, name="yt", tag="yt")
            yl = nc.sync.dma_start(
                out=yt[:, :n, :],
                in_=y1full[lo * 128:(lo + n) * 128, :].rearrange(
                    "(c p) d -> p c d", p=128))
            for dep in (cc1a, cc1b, z1):
                _add_dep_helper(yl.ins, dep.ins, sync=True, reason="b1 dense rhs")
            btg = gp.tile([128, NG1 * 128], BF, name="btg", tag="btg")
            nc.sync.dma_start(out=btg[:, :n * 128],
                              in_=b1t_d[:, lo * 128:(lo + n) * 128])
            for cc in range(n):
                ch = lo + cc
                first, last = ch == 0, ch == K1CH - 1
                nc.tensor.matmul(pseg4[:, 0:HID], btg[:, cc * 128:(cc + 1) * 128],
                                 yt[:, cc, 0:HID], start=first, stop=last)
                nc.tensor.matmul(pseg4[:, HID:YC], btg[:, cc * 128:(cc + 1) * 128],
                                 yt[:, cc, HID:YC], start=first, stop=last)
        x1sb4 = sp.tile([128, HID], F32, name="x1", tag="x1")
        emit_post(pseg4, HPC - 512, 1, x1sb4)
        for kc in range(4):
            tt = tp.tile([128, 128], F32, name="tp", tag="tp")
            nc.tensor.transpose(tt[:, :], x1sb4[:, kc * 128:(kc + 1) * 128], ident)
            nc.vector.tensor_copy(x1T[kc][:, 4 * 128:5 * 128], tt[:, :])

        for t in range(B1_PTILES - 1):
            pseg = segp.tile([128, YC], F32, name="seg", tag="seg")
            for k4 in range(CPT1):
                call = CPT1 * t + k4
                g = gp.tile([128, CHUNK // 128, YC], BF, name="g", tag="g")
                gi = nc.gpsimd.dma_gather(
                    g[:], y1full[:, :],
                    idx1[:, call * (CHUNK // 16):(call + 1) * (CHUNK // 16)],
                    CHUNK, CHUNK, YC, single_packet=False)
                for dep in (cc1a, cc1b, z1):
                    _add_dep_helper(gi.ins, dep.ins, sync=True, reason="gather1 deps")
                for s in range(SLOTS):
                    q = SLOTS * k4 + s
                    off = 127 - 2 * q
                    first = (k4 == 0 and s == 0)
                    last = (k4 == CPT1 - 1 and s == SLOTS - 1)
                    nc.tensor.matmul(pseg[:, 0:HID], strip1[:, off:off + 128],
                                     g[:, s, 0:HID], start=first, stop=last)
                    nc.tensor.matmul(pseg[:, HID:YC], strip1[:, off:off + 128],
                                     g[:, s, HID:YC], start=first, stop=last)
            rows = min(128, HPC - t * 128)
            x1sb = sp.tile([128, HID], F32, name="x1", tag="x1")
            emit_post(pseg, rows, 1, x1sb)
            for kc in range(4):
                tt = tp.tile([128, 128], F32, name="tp", tag="tp")
                nc.tensor.transpose(tt[:, :], x1sb[:, kc * 128:(kc + 1) * 128], ident)
                nc.vector.tensor_copy(x1T[kc][:, t * 128:(t + 1) * 128], tt[:, :])

        # ---------------- block 2: y2 production
        for rt in range(B1_PTILES):
            rows = min(128, HPC - rt * 128)
            y2_writes.append(emit_y(rows, 4, x1T, rt, V2, qv2, y2loc, rt * 128))

        cc2a = cc2b = nc.gpsimd.collective_compute(
            "AllGather", Alu.bypass, replica_groups=rg,
            ins=[y2loc[:, :]], outs=[y2full[0:MPAD, :]])
        for w in y2_writes:
            _add_dep_helper(cc2a.ins, w.ins, sync=True, reason="ag2 after y2 writes")

        # ---------------- block 2: hybrid — 14 tiles via dma_gather (GpSimd),
        # 6 tiles via dense incidence matmul (PE), running concurrently
        gp.release()
        xp.release()
        b2p = tc.alloc_tile_pool(name="b2p", bufs=1)
        g2p = tc.alloc_tile_pool(name="g2p", bufs=2)
        btp = tc.alloc_tile_pool(name="btp", bufs=1)
        y2sb = b2p.tile([128, K2CH, YC], BF, name="y2sb", tag="y2sb")
        yld = nc.sync.dma_start(
            out=y2sb[:],
            in_=y2full[0:K2CH * 128, :].rearrange("(c p) d -> p c d", p=128))
        for dep in (cc2a, cc2b, z2):
            _add_dep_helper(yld.ins, dep.ins, sync=True, reason="y2sb after ag2/zeros")

        def b2_gather_tile(t, gcall):
            pseg = segp.tile([128, YC], F32, name="seg", tag="seg")
            g = g2p.tile([128, CHUNK // 128, YC], BF, name="g", tag="g")
            gi = nc.gpsimd.dma_gather(
                g[:], y2full[:, :],
                idx2[:, gcall * (CHUNK // 16):(gcall + 1) * (CHUNK // 16)],
                CHUNK, CHUNK, YC, single_packet=False)
            for dep in (cc2a, cc2b, z2):
                _add_dep_helper(gi.ins, dep.ins, sync=True, reason="gather2 deps")
            for s in range(SLOTS):
                off = 127 - 8 * s
                first, last = s == 0, s == SLOTS - 1
                nc.tensor.matmul(pseg[:, 0:HID], strip2[:, off:off + 128],
                                 g[:, s, 0:HID], start=first, stop=last)
                nc.tensor.matmul(pseg[:, HID:YC], strip2[:, off:off + 128],
                                 g[:, s, HID:YC], start=first, stop=last)
            return pseg

        def b2_dense_tile(t, dt):
            bt = btp.tile([128, K2CH * 128], BF, name="bt", tag="bt")
            nc.sync.dma_start(out=bt[:], in_=b2t_d[dt * 128:(dt + 1) * 128, :])
            pseg = segp.tile([128, YC], F32, name="seg", tag="seg")
            for hc in range(K2CH):
                first, last = hc == 0, hc == K2CH - 1
                nc.tensor.matmul(pseg[:, 0:HID], bt[:, hc * 128:(hc + 1) * 128],
                                 y2sb[:, hc, 0:HID], start=first, stop=last)
                nc.tensor.matmul(pseg[:, HID:YC], bt[:, hc * 128:(hc + 1) * 128],
                                 y2sb[:, hc, HID:YC], start=first, stop=last)
            return pseg

        # interleave: (g g d) x 6 + (g g) -> gather tiles get node-tiles 0..13,
        # dense tiles get node-tiles 14..19; emission order mixes them so PE
        # dense work fills the gaps while GpSimd streams gathers.
        order = []
        gi_, di_ = 0, 0
        for grp in range(6):
            order += [('g', gi_), ('g', gi_ + 1), ('d', di_)]
            gi_ += 2
            di_ += 1
        order += [('g', 12), ('g', 13)]
        for kind, j in order:
            if kind == 'g':
                t = j
                pseg = b2_gather_tile(t, j)
            else:
                t = G2T + j
                pseg = b2_dense_tile(t, j)
            rows = min(128, NPC - t * 128)
            osb = sp.tile([128, HID], F32, name="osb", tag="osb")
            emit_post(pseg, rows, 2, osb)
            nc.scalar.dma_start(out=out_d[t * 128:t * 128 + rows, :], in_=osb[:rows, :])

        for p in (btp, g2p, b2p, tp, mmp, segp, st, sp, wp):
            p.release()

    nc.compile()
    return nc


# ----------------------------------------------------------------- entry

def kernel(**inputs):
    from concourse.bass_utils import run_bass_kernel_spmd

    in_maps = _host_prep(inputs)
    triv_ln = tuple(
        bool(np.all(np.asarray(inputs[g]) == 1.0) and np.all(np.asarray(inputs[b]) == 0.0))
        for g, b in (('ve_ln0_g', 've_ln0_b'), ('ve_ln1_g', 've_ln1_b'),
                     ('ev_ln0_g', 'ev_ln0_b'), ('ev_ln1_g', 'ev_ln1_b')))
    triv_b = tuple(
        bool(np.all(np.asarray(inputs[b1]) == 0.0) and np.all(np.asarray(inputs[b2]) == 0.0))
        for b1, b2 in (('ve_b1', 've_b2'), ('ev_b1', 'ev_b2')))
    key = (triv_ln, triv_b)
    if _CACHE.get('key') != key:
        _CACHE['nc'] = _build(triv_ln, triv_b)
        _CACHE['key'] = key
    nc = _CACHE['nc']
    res = run_bass_kernel_spmd(nc, in_maps, core_ids=list(range(N_CORES)))
    out = np.vstack([res.results[c]['out'] for c in range(N_CORES)])
    return out.astype(np.float32)


if __name__ == '__main__':
    data = dict(np.load('/root/problem/work/inputs.npz'))
    got = kernel(**data)
    exp = np.load('/root/problem/work/expected.npy')
    num = np.linalg.norm(got - exp)
    den = np.linalg.norm(exp)
    print(f"rel_fro={num / den:.3e} maxabs={np.abs(got - exp).max():.3e}")

